# revision 18
# baseline (speedup 1.0000x reference)
"""TRN2 Bass kernel for nn_Estimator_4037269258477 (gnn_message_passing).

Strategy
--------
Data-parallel over batch: 16 images -> 8 cores x 2 images. Each core runs the
full two-branch message-passing graph on its 2 images.

All on-chip activations live in a space-to-depth (s=4) layout: a [C, 64, 64]
image becomes rows (q, c) with q = uy*4+ux in {0..15}, spatial [16, 16].
A 7x7 conv then becomes 9 taps (ty, tx in {-1,0,1}) of dense matmuls with
K = 16*C_in per tap (chunked to 128 partitions), M = 16*C_out, N = 512
(2 images x 256 pixels). Tap spatial shifts are pure access-pattern offsets:
x-direction via a zero-padded 18-column layout, y-direction via row trims
(PSUM per-element has_written semantics make trimmed accumulation correct).

Matmuls run in fp32r (full PE rate at N>=256, ~1.5e-4 rel err). Conv weights
are host-expanded into the s2d block layout, shipped bf16, DMA-cast to fp32r.
The 26 pre-einsums (1x1 conv 256->16) run as a prologue with 8 joints packed
into M=128, round-trip through DRAM, and are accumulated into per-label
message boxes via accumulate-DMA. BN is folded into per-partition scale/bias
pairs applied by the scalar engine together with ReLU. The final heatmap
channel reduction folds its (trivial) BN+ReLU into the weights (validity
asserted on host) and accumulates per-joint [16, 512] tiles.

Host does all layout (un)shuffling; the device never converts layouts.
"""

import os

import numpy as np
import ml_dtypes

import concourse.bacc as bacc
import concourse.mybir as mybir
from concourse.tile import TileContext
from concourse.bass_utils import run_bass_kernel_spmd

F32 = mybir.dt.float32
F32R = mybir.dt.float32r
BF16 = mybir.dt.bfloat16
BF16_NP = ml_dtypes.bfloat16
LAST_EXEC_NS = None

# ---------------------------------------------------------------- graph spec
PASS_A = ["1", "13", "12", "11", "10", "5", "4", "7", "9", "6", "8", "2", "3"]
PASS_B = ["2", "3", "6", "8", "7", "9", "4", "5", "10", "11", "12", "13", "1"]
REL_A = {
    "1": ["2", "3", "7", "9", "13"], "2": [], "3": [], "4": ["7", "9"],
    "5": ["4"], "6": ["2"], "7": ["6"], "8": ["3"], "9": ["8"], "10": ["5"],
    "11": ["10"], "12": ["11"], "13": ["12"],
}
REL_B = {
    "1": [], "2": ["1", "6"], "3": ["1", "8"], "4": ["5"], "5": ["10"],
    "6": ["7"], "7": ["1", "4"], "8": ["9"], "9": ["1", "4"], "10": ["11"],
    "11": ["12"], "12": ["13"], "13": ["1"],
}
BATCH, CIN, CH, H, W = 16, 256, 16, 64, 64
N_CORES = 8
BL = BATCH // N_CORES          # images per core = 2
S = 4                          # space-to-depth factor
Q = S * S                      # 16 quadrant-phases
YS, XS = H // S, W // S        # 16 x 16 transformed spatial
XP = XS + 2                    # padded width (zero col at each end)
YP = YS + 2                    # padded height
NF = BL * YS * XS              # dense free size = 512
NFP = BL * YP * XP             # padded free size = 648*2
TAPS = [(ty, tx) for ty in (-1, 0, 1) for tx in (-1, 0, 1)]
TAPS.sort(key=lambda t: (abs(t[0]) + abs(t[1]), t))  # (0,0) first
EPS = 1e-5

# per-branch schedule: list of (slot, label, edge pass idxs, tgt labels, inter idx)
def _schedule():
    sched = []
    cnt = 0
    for br, (order, rel) in enumerate(((PASS_A, REL_A), (PASS_B, REL_B))):
        inter_start = 32 if br == 0 else 45
        for i, lab in enumerate(order):
            slot = br * 13 + i
            tgts = rel[lab]
            passes = [(cnt + k, t) for k, t in enumerate(tgts)]
            cnt += len(tgts)
            sched.append((slot, br, lab, passes, inter_start + i))
    return sched

SCHED = _schedule()
N_JG = 4  # pre-einsum joint groups of 8

# ------------------------------------------------------------- host layouts
def s2d_rows(t):
    """[img, C, 64, 64] -> [img, Q*C rows, 16, 16]; row = q*C + c."""
    b, c = t.shape[0], t.shape[1]
    v = t.reshape(b, c, YS, S, XS, S)            # y = 4Y+uy, x = 4X+ux
    v = v.transpose(0, 3, 5, 1, 2, 4)            # [img, uy, ux, c, Y, X]
    return v.reshape(b, Q * c, YS, XS)

def inv_s2d(rows):
    """[img, Q*C, 16, 16] -> [img, C, 64, 64]."""
    b = rows.shape[0]
    c = rows.shape[1] // Q
    v = rows.reshape(b, S, S, c, YS, XS).transpose(0, 3, 4, 1, 5, 2)
    return v.reshape(b, c, H, W)

def dense_f(rows):
    """[img, R, 16, 16] -> [R, img*256 + Y*16 + X]."""
    return np.ascontiguousarray(rows.transpose(1, 0, 2, 3)).reshape(
        rows.shape[1], -1
    )

def conv_w_s2d(w):
    """w [O, C, 7, 7] -> w_exp[tap9][row=(q_in, c)][col=(q_out, o)] f32."""
    O, C = w.shape[0], w.shape[1]
    out = np.zeros((9, Q * C, Q * O), np.float32)
    for t, (ty, tx) in enumerate(TAPS):
        for uy_i in range(S):
            for ux_i in range(S):
                for uy_o in range(S):
                    for ux_o in range(S):
                        dy = S * ty + uy_i - uy_o + 3
                        dx = S * tx + ux_i - ux_o + 3
                        if 0 <= dy < 7 and 0 <= dx < 7:
                            qi = uy_i * S + ux_i
                            qo = uy_o * S + ux_o
                            out[t, qi * C : qi * C + C, qo * O : qo * O + O] = (
                                w[:, :, dy, dx].T
                            )
    return out

def bn_fold(g, b, m, v):
    inv = g / np.sqrt(v + EPS)
    return inv.astype(np.float32), (b - m * inv).astype(np.float32)

# ------------------------------------------------------------ device kernel
DBG_SLOTS = int(os.environ.get("K_SLOTS", "26"))
DBG_SKIP_ACCUM = bool(int(os.environ.get("K_SKIP_ACCUM", "0")))
DBG_SKIP_HEAT = bool(int(os.environ.get("K_SKIP_HEAT", "0")))
DBG_SKIP_PRE = bool(int(os.environ.get("K_SKIP_PRE", "0")))


def build_program():
    nc = bacc.Bacc("TRN2", target_bir_lowering=False, debug=False)
    dp = nc.declare_dram_parameter

    xs2d_d = dp("xs2d", [128, 2 * Q * NF], F32, isOutput=False)
    prew_d = dp("prew", [N_JG, 2, 128, 128], F32, isOutput=False)
    wdt = F32 if W_F32 else BF16
    w1x_d = dp("w1x", [58, 128, 9 * 2 * 128], wdt, isOutput=False)
    w2x_d = dp("w2x", [58, 128, 9 * 2 * 128], wdt, isOutput=False)
    heatw_d = dp("heatw", [26, 2, 128, 16], F32, isOutput=False)
    scb_d = dp("scb", [128, 58 * 2], F32, isOutput=False)
    prebn_d = dp("prebn", [128, 26 * 2], F32, isOutput=False)
    ahxs_d = dp("ahxs", [26, 2, 128, NF], F32, isOutput=False)
    inter_d = dp("inter_o", [26, 2, 128, NF], F32, isOutput=True)
    heat_d = dp("heat_o", [16, 13 * NF], F32, isOutput=True)

    preo_d = nc.dram_tensor("preo", [26, 2, 128, NF], F32)

    with TileContext(nc) as tc:
        with (
            tc.tile_pool(name="const", bufs=1) as cpool,
            tc.tile_pool(name="psA", bufs=2, space="PSUM") as psA,   # conv1 / pre
            tc.tile_pool(name="psB", bufs=4, space="PSUM") as psB,   # conv2
            tc.tile_pool(name="psH", bufs=2, space="PSUM") as psH,   # heat / pre
        ):
            # ---- constants
            scb_t = cpool.tile([128, 58 * 2], F32, tag="scb")
            nc.sync.dma_start(out=scb_t[:], in_=scb_d[:])
            prebn_t = cpool.tile([128, 26 * 2], F32, tag="prebn")
            nc.sync.dma_start(out=prebn_t[:], in_=prebn_d[:])
            heatw_t = cpool.tile([128, 26 * 2 * 16], F32R, tag="heatw")
            nc.gpsimd.dma_start(
                out=heatw_t[:].rearrange("p (s k c) -> p s k c", s=26, k=2, c=16),
                in_=heatw_d[:].rearrange("s k p c -> p s k c"),
            )

            # ---- prologue: pre-einsums, 8 joints packed in M per q
            plg = tc.tile_pool(name="plg", bufs=1)
            _dummy = 0
            xpool = plg.__enter__()
            if DBG_SKIP_PRE:
                jgs = []
            else:
                jgs = list(range(N_JG))
            xs_t = xpool.tile([128, 2 * Q * NF], F32R, tag="xs")
            nc.gpsimd.dma_start(out=xs_t[:], in_=xs2d_d[:])
            xs_v = xs_t[:].rearrange("p (q k f) -> p q k f", q=Q, k=2, f=NF)
            def emit_jg(jg):
                pw_t = xpool.tile([128, 2 * 128], F32R, tag="prew")
                nc.gpsimd.dma_start(
                    out=pw_t[:].rearrange("p (k m) -> p k m", k=2),
                    in_=prew_d[jg, :, :, :].rearrange("k p m -> p k m"),
                )
                st_t = xpool.tile([128, Q * NF], F32, tag="stage")
                for q in range(Q):
                    pp = psH.tile([128, NF], F32, tag="psh")
                    for kc in range(2):
                        nc.tensor.matmul(
                            pp[:],
                            pw_t[:, kc * 128 : kc * 128 + 128],
                            xs_v[:, q, kc, :],
                            start=(kc == 0),
                            stop=(kc == 1),
                        )
                    nc.vector.tensor_copy(
                        out=st_t[:, q * NF : (q + 1) * NF], in_=pp[:]
                    )
                st_v = st_t[:].rearrange(
                    "(j o) (q f) -> j o q f", j=8, o=16, q=Q, f=NF
                )
                for jl in range(8):
                    slot = jg * 8 + jl
                    if slot >= 26:
                        break
                    for kc in range(2):
                        nc.sync.dma_start(
                            out=preo_d[slot, kc, :, :].rearrange(
                                "(q o) f -> o q f", q=8, o=16
                            ),
                            in_=st_v[jl, :, 8 * kc : 8 * kc + 8, :],
                        )

            for _g in jgs:
                emit_jg(_g)

            # ---- main: joints in slot order
            mw = tc.tile_pool(name="mainp", bufs=2)
            wpool = mw.__enter__()
            mm = tc.tile_pool(name="mbox", bufs=14)
            mpool = mm.__enter__()
            ma = tc.tile_pool(name="act", bufs=2)
            apool = ma.__enter__()
            mo = tc.tile_pool(name="out", bufs=3)
            opool = mo.__enter__()
            mbox = {}       # (br, lab) -> [tile kc0, tile kc1]

            def get_mbox(br, lab, slot_for_load):
                key = (br, lab)
                if key not in mbox:
                    tiles = []
                    for kc in range(2):
                        t = mpool.tile([128, NF], F32, tag="mbox")
                        nc.sync.dma_start(
                            out=t[:], in_=ahxs_d[slot_for_load, kc, :, :]
                        )
                        if not DBG_SKIP_ACCUM:
                            nc.gpsimd.dma_start(
                                out=t[:],
                                in_=preo_d[slot_for_load, kc, :, :],
                                accum_op=mybir.AluOpType.add,
                            )
                        tiles.append(t)
                    mbox[key] = tiles
                return mbox[key]

            slot_of = {}
            for slot, br, lab, passes, inter_idx in SCHED:
                slot_of[(br, lab)] = slot

            for slot, br, lab, passes, inter_idx in SCHED:
                if slot >= DBG_SLOTS:
                    break
                mb = get_mbox(br, lab, slot)
                # result = relu(scale * mbox + bias), into x-padded layout
                res = []
                for kc in range(2):
                    r = apool.tile([128, NFP], F32R, tag=f"res{kc}")
                    r_v = r[:].rearrange(
                        "p (i y x) -> p i y x", i=BL, y=YP, x=XP
                    )
                    nc.gpsimd.memset(
                        r_v[:, :, 0 : YP : YP - 1, :].bitcast(F32), 0.0
                    )
                    nc.gpsimd.memset(
                        r_v[:, :, 1 : 1 + YS, 0 : XP : XP - 1].bitcast(F32), 0.0
                    )
                    mb_v = mb[kc][:].rearrange(
                        "p (i y x) -> p i y x", i=BL, y=YS, x=XS
                    )
                    nc.scalar.activation(
                        out=r_v[:, :, 1 : 1 + YS, 1 : 1 + XS],
                        in_=mb_v[:, :, :, :],
                        func=mybir.ActivationFunctionType.Relu,
                        scale=prebn_t[:, slot * 2 : slot * 2 + 1],
                        bias=prebn_t[:, slot * 2 + 1 : slot * 2 + 2],
                    )
                    res.append(r)
                del mb
                mbox.pop((br, lab))

                # conv passes
                all_passes = [(pi, tgt) for pi, tgt in passes] + [
                    (inter_idx, None)
                ]
                for pi, tgt in all_passes:
                    w1_t = wpool.tile([128, 9 * 2 * 128], F32R, tag="w1")
                    nc.gpsimd.dma_start(out=w1_t[:], in_=w1x_d[pi, :, :])
                    w2_t = wpool.tile([128, 9 * 2 * 128], F32R, tag="w2")
                    nc.gpsimd.dma_start(out=w2_t[:], in_=w2x_d[pi, :, :])
                    # conv1: 9 taps x 2 kc -> psum [128=(q,o), 512]
                    p1 = psA.tile([128, NF], F32, tag="c1")
                    p1_v = p1[:].rearrange(
                        "p (i y x) -> p i y x", i=BL, y=YS, x=XS
                    )
                    first = True
                    for t, (ty, tx) in enumerate(TAPS):
                        for kc in range(2):
                            r_v = res[kc][:].rearrange(
                                "p (i y x) -> p i y x", i=BL, y=YP, x=XP
                            )
                            nc.tensor.matmul(
                                p1_v[:, :, :, :],
                                w1_t[
                                    :, (t * 2 + kc) * 128 : (t * 2 + kc + 1) * 128
                                ],
                                r_v[
                                    :, :, 1 + ty : 1 + ty + YS,
                                    1 + tx : 1 + tx + XS
                                ],
                                start=first,
                                stop=(t == 8 and kc == 1),
                            )
                            first = False
                    # relu-bn evac into padded c1out
                    c1o = apool.tile([128, NFP], F32R, tag="c1o")
                    c1o_v = c1o[:].rearrange(
                        "p (i y x) -> p i y x", i=BL, y=YP, x=XP
                    )
                    nc.gpsimd.memset(
                        c1o_v[:, :, 0 : YP : YP - 1, :].bitcast(F32), 0.0
                    )
                    nc.gpsimd.memset(
                        c1o_v[:, :, 1 : 1 + YS, 0 : XP : XP - 1].bitcast(F32), 0.0
                    )
                    nc.scalar.activation(
                        out=c1o_v[:, :, 1 : 1 + YS, 1 : 1 + XS],
                        in_=p1_v[:, :, :, :],
                        func=mybir.ActivationFunctionType.Relu,
                        scale=scb_t[:, pi * 2 : pi * 2 + 1],
                        bias=scb_t[:, pi * 2 + 1 : pi * 2 + 2],
                    )
                    # conv2: per M-group g -> psum [128=(q_g, c'), 512]
                    for g in range(2):
                        p2 = psB.tile([128, NF], F32, tag="c2")
                        p2_v = p2[:].rearrange(
                            "p (i y x) -> p i y x", i=BL, y=YS, x=XS
                        )
                        first = True
                        for t, (ty, tx) in enumerate(TAPS):
                            nc.tensor.matmul(
                                p2_v[:, :, :, :],
                                w2_t[
                                    :, (t * 2 + g) * 128 : (t * 2 + g + 1) * 128
                                ],
                                c1o_v[
                                    :, :, 1 + ty : 1 + ty + YS,
                                    1 + tx : 1 + tx + XS
                                ],
                                start=first,
                                stop=(t == 8),
                            )
                            first = False
                        if tgt is None:
                            ob = opool.tile([128, NF], F32, tag="outb")
                            if INTER_ACT:
                                nc.scalar.copy(out=ob[:], in_=p2[:])
                            else:
                                nc.vector.tensor_copy(out=ob[:], in_=p2[:])
                            nc.sync.dma_start(
                                out=inter_d[slot, g, :, :], in_=ob[:]
                            )
                        else:
                            tslot = slot_of[(br, tgt)]
                            if tslot >= DBG_SLOTS:
                                continue
                            tmb = get_mbox(br, tgt, tslot)
                            nc.vector.tensor_add(
                                out=tmb[g][:], in0=tmb[g][:], in1=p2[:]
                            )

                # heat partial: 2 matmuls over kc chunks
                if DBG_SKIP_HEAT:
                    continue
                c = int(lab) - 1
                ph_t = psH.tile([128, NF], F32, tag="psh")
                ph = ph_t[0:16, :]
                for kc in range(2):
                    r_v = res[kc][:].rearrange(
                        "p (i y x) -> p i y x", i=BL, y=YP, x=XP
                    )
                    hw_v = heatw_t[:].rearrange(
                        "p (s k c) -> p s k c", s=26, k=2, c=16
                    )
                    nc.tensor.matmul(
                        ph[:],
                        hw_v[:, slot, kc, :],
                        r_v[:, :, 1 : 1 + YS, 1 : 1 + XS],
                        start=(kc == 0),
                        stop=(kc == 1),
                    )
                hb = opool.tile([16, NF], F32, tag="heatb")
                nc.vector.tensor_copy(out=hb[:], in_=ph[:])
                if br == 0:
                    nc.sync.dma_start(
                        out=heat_d[:, c * NF : (c + 1) * NF], in_=hb[:]
                    )
                else:
                    nc.gpsimd.dma_start(
                        out=heat_d[:, c * NF : (c + 1) * NF],
                        in_=hb[:],
                        accum_op=mybir.AluOpType.add,
                    )

            mo.__exit__(None, None, None)
            ma.__exit__(None, None, None)
            mm.__exit__(None, None, None)
            mw.__exit__(None, None, None)
            plg.__exit__(None, None, None)

    nc.finalize()
    return nc

# --------------------------------------------------------------- host entry
def _prep_core_inputs(core, x, ahead_msg, prew_h, w1x_h, w2x_h, heatw_h,
                      scb_h, prebn_h):
    i0 = core * BL
    xs = s2d_rows(x[i0 : i0 + BL])               # [2, 4096, 16, 16]
    # xs2d[c_local, (q, kc, img, Y, X)]: row (q, c) with c = kc*128 + c_local
    v = xs.reshape(BL, Q, CIN, YS, XS)           # row = q*CIN + c
    v = v.reshape(BL, Q, 2, 128, YS, XS).transpose(3, 1, 2, 0, 4, 5)
    xs2d = np.ascontiguousarray(v).reshape(128, 2 * Q * NF)

    # ahxs[slot, kc, row=(q_l, c'), img*256+Y*16+X]
    ahxs = np.zeros((26, 2, 128, NF), np.float32)
    for slot, br, lab, _p, _i in SCHED:
        a = s2d_rows(ahead_msg[br, int(lab) - 1, i0 : i0 + BL])  # [2,256,16,16]
        a = dense_f(a)                            # [256, 512]
        ahxs[slot] = a.reshape(2, 128, NF)
    return xs2d, ahxs

def kernel(x, ahead_msg, pre_w, pre_bn_g, pre_bn_b, pre_bn_m, pre_bn_v,
           aft_bn_g, aft_bn_b, aft_bn_m, aft_bn_v, aft_w,
           pass_w1, pass_w2, pass_bn_g, pass_bn_b, pass_bn_m, pass_bn_v):
    x = np.asarray(x, np.float32)
    ahead_msg = np.asarray(ahead_msg, np.float32)

    # ---- shared (replicated) weight prep
    # pre weights: prew[jg, kc, c_local, (j_l, o)]
    prew_h = np.zeros((N_JG, 2, 128, 128), np.float32)
    for jg in range(N_JG):
        for jl in range(8):
            s = jg * 8 + jl
            if s >= 26:
                break
            w = np.asarray(pre_w[s], np.float32)          # [16, 256]
            for kc in range(2):
                prew_h[jg, kc, :, jl * 16 : jl * 16 + 16] = w[
                    :, kc * 128 : kc * 128 + 128
                ].T

    # conv weights expanded to s2d layout, bf16
    wdt_np = np.float32 if W_F32 else BF16_NP
    w1x_h = np.zeros((58, 128, 9 * 2 * 128), wdt_np)
    w2x_h = np.zeros((58, 128, 9 * 2 * 128), wdt_np)
    for p in range(58):
        e1 = conv_w_s2d(np.asarray(pass_w1[p], np.float32))  # [9, 256, 128]
        w1x_h[p] = (
            e1.reshape(9, 2, 128, 128).transpose(2, 0, 1, 3).reshape(128, -1)
        )
        e2 = conv_w_s2d(np.asarray(pass_w2[p], np.float32))  # [9, 128, 256]
        # M-groups: g selects out chunk (q in [8g, 8g+8))
        e2g = e2.reshape(9, 128, 2, 128).transpose(0, 2, 1, 3)  # [9,2,128,128]
        w2x_h[p] = e2g.transpose(2, 0, 1, 3).reshape(128, -1)

    # bn folds
    scb_h = np.zeros((128, 58 * 2), np.float32)
    for p in range(58):
        sc, bi = bn_fold(np.asarray(pass_bn_g[p], np.float32),
                         np.asarray(pass_bn_b[p], np.float32),
                         np.asarray(pass_bn_m[p], np.float32),
                         np.asarray(pass_bn_v[p], np.float32))  # [8]
        scb_h[:, 2 * p] = np.tile(sc, Q)
        scb_h[:, 2 * p + 1] = np.tile(bi, Q)

    prebn_h = np.zeros((128, 26 * 2), np.float32)
    for s in range(26):
        sc, bi = bn_fold(np.asarray(pre_bn_g[s], np.float32),
                         np.asarray(pre_bn_b[s], np.float32),
                         np.asarray(pre_bn_m[s], np.float32),
                         np.asarray(pre_bn_v[s], np.float32))  # [16]
        # chunk rows are (q_l, c'); scale depends only on c' -> same for both
        prebn_h[:, 2 * s] = np.tile(sc, 8)
        prebn_h[:, 2 * s + 1] = np.tile(bi, 8)

    # heat weights with folded aft bn + relu-fold validity check
    aft_sc, aft_bi = bn_fold(np.asarray(aft_bn_g, np.float32),
                             np.asarray(aft_bn_b, np.float32),
                             np.asarray(aft_bn_m, np.float32),
                             np.asarray(aft_bn_v, np.float32))  # [13, 32]
    assert np.all(aft_sc > 0) and np.all(np.abs(aft_bi) < 1e-30), (
        "heat relu-fold invalid for these bn params"
    )
    afw = np.asarray(aft_w, np.float32) * aft_sc              # [13, 32]
    heatw_h = np.zeros((26, 2, 128, 16), np.float32)
    for slot, br, lab, _p, _i in SCHED:
        c = int(lab) - 1
        w = afw[c, br * 16 : br * 16 + 16]                    # [16] per channel
        for kc in range(2):
            for ql in range(8):
                q = 8 * kc + ql
                heatw_h[slot, kc, ql * 16 : ql * 16 + 16, q] = w

    nc = build_program()

    in_maps = []
    for core in range(N_CORES):
        xs2d, ahxs = _prep_core_inputs(core, x, ahead_msg, prew_h, w1x_h,
                                       w2x_h, heatw_h, scb_h, prebn_h)
        in_maps.append(dict(
            xs2d=xs2d, prew=prew_h, w1x=w1x_h, w2x=w2x_h, heatw=heatw_h,
            scb=scb_h, prebn=prebn_h, ahxs=ahxs,
        ))

    trace = bool(int(os.environ.get("KERNEL_TRACE", "0")))
    res = run_bass_kernel_spmd(nc, in_maps, list(range(N_CORES)), trace=trace)
    global LAST_EXEC_NS
    LAST_EXEC_NS = res.exec_time_ns

    heat = np.zeros((BATCH, 13, H, W), np.float32)
    inter_a = np.zeros((BATCH, 13, CH, H, W), np.float32)
    inter_b = np.zeros((BATCH, 13, CH, H, W), np.float32)
    for core in range(N_CORES):
        i0 = core * BL
        out = res.results[core]
        ho = out["heat_o"].reshape(16, 13, BL, YS, XS)        # [q, c, img, Y, X]
        hrows = ho.transpose(2, 1, 0, 3, 4).reshape(BL, 13 * Q, YS, XS)
        # rows within each c are q-major with C=1: inv_s2d wants row=(q, c=1)
        for c in range(13):
            heat[i0 : i0 + BL, c] = inv_s2d(
                hrows[:, c * Q : (c + 1) * Q].reshape(BL, Q, YS, XS)
            )[:, 0]
        io = out["inter_o"].reshape(26, 2 * 128, BL, YS, XS)
        for slot, br, lab, _p, _i in SCHED:
            c = int(lab) - 1
            rows = io[slot].transpose(1, 0, 2, 3)             # [img, 256, Y, X]
            dst = inter_a if br == 0 else inter_b
            dst[i0 : i0 + BL, c] = inv_s2d(rows)
    return heat, inter_a, inter_b
